# revision 11
# baseline (speedup 1.0000x reference)
"""Trainium2 Bass kernel for ClassLinearWithLORA (moe_routing).

Computes out = x @ W.T + b + gates[-1] * (alpha * (x @ A[-1]) @ B_lora[-1])
(the torch loop overwrites out_lora each class iteration, so only the last
class adapter contributes).

Strategy:
  - Data-parallel shard of the 8192 (B*S) rows across 8 NeuronCores
    (1024 rows/core); W/b and the rank-16 LoRA stacks are replicated.
  - Matmuls run in bf16 (1 cycle/row on the PE, same rate as fp32r, but
    HALF the HBM/DMA traffic, which removes PE starvation stalls). PSUM
    accumulation stays fp32; the output is stored bf16 and widened on host.
  - Formulation: psum[r128, o512] = sum_k xT[k][:, r].T @ WT[k][:, o]
    accumulated over 8 K-tiles, plus ONE augmented K=17 matmul per tile
    that adds both the LoRA rank-16 update and the bias:
       lhsT_aug = [ (g * (x @ A)).T ; ones ]  (17 x r)
       rhs_aug  = [ alpha * B_lora[-1] ; b ]  (17 x o)
  - Startup: the per-core rows are rotated by 512 on the host so the first
    K-chunk of xT splits into a tiny 32KB piece (Pool/SWDGE ring, which
    does not contend for the shared HWDGE descriptor generator) plus one
    big ACT-ring piece; wt block 0's first K-slice leads the SP ring.
    All three first DMAs land ~2.6-3.0us and the PE starts at ~2.9us.
  - Tail: the final tile runs as [384, 128] column sub-tiles so the last
    store's copy+DMA chain is as short as possible after the PE finishes.
"""

import numpy as np
import ml_dtypes

import concourse.bacc as bacc
import concourse.mybir as mybir
import concourse.tile as tile
from concourse.bass_utils import run_bass_kernel_spmd

F32 = mybir.dt.float32
BF16 = mybir.dt.bfloat16
NP_BF16 = ml_dtypes.bfloat16

N_CORES = 8
B, S, D_IN, D_OUT, R_LORA = 4, 2048, 1024, 4096, 16
ROWS = B * S                  # 8192
R_CORE = ROWS // N_CORES      # 1024 rows per core
KT = D_IN // 128              # 8 K-tiles of 128
NB = 512                      # moving free dim per matmul (PSUM bank limit)
OB = D_OUT // NB              # 8 output blocks
RT = R_CORE // 128            # 8 row tiles per core
KA = R_LORA + 1               # augmented contraction (16 LoRA + 1 bias)
ROT = 512                     # host-side row rotation (see module docstring)


AUG_FIRST = True

def _build(
    xt_chunks: int = 8,
    xt_engine: str = "scalar",
    wt_bufs: int = 3,
    psum_bufs: int = 8,
    out_bufs: int = 4,
    wt0_split: int = 8,
    wt_split: int = 2,
    tail_split: int = 384,
):
    nc = bacc.Bacc(None, target_bir_lowering=False)

    x_d = nc.dram_tensor("xt", [128, KT, R_CORE], BF16, kind="ExternalInput")
    w_d = nc.dram_tensor("wt", [128, OB, KT, NB], BF16, kind="ExternalInput")
    a_d = nc.dram_tensor("a_lora", [128, KT, R_LORA], BF16, kind="ExternalInput")
    rhs_d = nc.dram_tensor("aug_rhs", [KA, D_OUT], BF16, kind="ExternalInput")
    g_d = nc.dram_tensor("g_rep", [R_LORA, R_CORE], F32, kind="ExternalInput")
    one_d = nc.dram_tensor("ones_row", [1, R_CORE], BF16, kind="ExternalInput")
    out_d = nc.dram_tensor("out", [R_CORE, D_OUT], BF16, kind="ExternalOutput")

    with tile.TileContext(nc) as tc:
        with (
            tc.tile_pool(name="resident", bufs=1) as res,
            tc.tile_pool(name="wpool", bufs=wt_bufs) as wpool,
            tc.tile_pool(name="opool", bufs=out_bufs) as opool,
            tc.tile_pool(name="psum", bufs=psum_bufs, space="PSUM") as psum,
        ):
            # ---- resident loads -------------------------------------------------
            # Ring assignment at t=0 (every early DMA pays ~630ns on the
            # shared HWDGE device, so each ring leads with what the PE needs
            # first):
            #   SP:   wt0 k-slice 0, a_lora, wt0 k-slices 1..7, steady wt
            #   ACT:  xt k0 cols 128:1024, xt chunks k=1..7, output stores
            #   Pool: xt k0 cols 0:128 (SWDGE, no HWDGE contention), g/rhs/ones
            ld = getattr(nc, xt_engine)
            a_sb = res.tile([128, KT, R_LORA], BF16)
            nc.sync.dma_start(a_sb[:], a_d.ap())
            wt0 = wpool.tile([128, KT, NB], BF16, tag="wt")
            kh = KT // wt0_split
            for h in range(wt0_split):
                nc.sync.dma_start(
                    wt0[:, h * kh : (h + 1) * kh, :],
                    w_d.ap()[:, 0, h * kh : (h + 1) * kh, :],
                )
            xt = res.tile([128, KT, R_CORE], BF16)
            hr = R_CORE // 2
            ld.dma_start(xt[:, 0, 0:hr], x_d.ap()[:, 0, 0:hr])
            ld.dma_start(xt[:, 0, hr:R_CORE], x_d.ap()[:, 0, hr:R_CORE])
            kc = KT // xt_chunks
            for k in range(1, xt_chunks):
                ld.dma_start(
                    xt[:, k * kc : (k + 1) * kc, :],
                    x_d.ap()[:, k * kc : (k + 1) * kc, :],
                )
            # g/rhs/ones are not consumed until the gate multiply and first
            # aug matmul (~15us in) — queue them on the ACT ring behind the
            # xt chunks so the SP ring streams wt blocks uninterrupted
            g_sb = res.tile([R_LORA, R_CORE], F32)
            ld.dma_start(g_sb[:], g_d.ap())
            rhs_sb = res.tile([KA, D_OUT], BF16)
            ld.dma_start(rhs_sb[:], rhs_d.ap())
            lora_aug = res.tile([KA, R_CORE], BF16)
            ld.dma_start(lora_aug[R_LORA : R_LORA + 1, :], one_d.ap())

            def emit_epilogue(ps, rt, ob):
                """Close psum tile: copy to SBUF, then store. For the last
                o-block, split copy+store in halves across both HWDGE rings
                (the SP ring is load-free by then) to shorten the tail chain."""
                o_sb = opool.tile([128, NB], BF16, tag="o_sb", name=f"o_{ob}_{rt}")
                orow = out_d.ap()[rt * 128 : (rt + 1) * 128, ob * NB : (ob + 1) * NB]
                if ob == OB - 1:
                    h = NB // 2
                    nc.vector.tensor_copy(o_sb[:, 0:h], ps[:, 0:h])
                    nc.scalar.dma_start(orow[:, 0:h], o_sb[:, 0:h])
                    nc.vector.tensor_copy(o_sb[:, h:NB], ps[:, h:NB])
                    nc.sync.dma_start(orow[:, h:NB], o_sb[:, h:NB])
                else:
                    nc.vector.tensor_copy(o_sb[:], ps[:])
                    nc.scalar.dma_start(orow[:], o_sb[:])

            def emit_aug(ps, rt, ob, start, stop):
                # rank-16 LoRA update + bias in one K=17 matmul
                nc.tensor.matmul(
                    ps[:],
                    lora_aug[:, rt * 128 : (rt + 1) * 128],
                    rhs_sb[:, ob * NB : (ob + 1) * NB],
                    start=start,
                    stop=stop,
                )

            # ---- prologue: ob=0 interleaved with the LoRA first matmul ---------
            # Per K-chunk: 2 lora matmuls plus 6 of the 8 ob=0 row tiles
            # (2 lora + 6 main psum tiles = 8 banks); rt=6,7 run densely
            # afterwards. k=0 order follows DMA arrival: cols 0:128 (Pool,
            # ~2.8us) -> 128:1024 (ACT, ~3.0us) -> a_lora (SP, ~3.6us).
            NRB = R_CORE // NB  # lora row blocks
            ps_l = [psum.tile([R_LORA, NB], F32, tag="ps", name=f"psl{rb}") for rb in range(NRB)]
            ps0 = [psum.tile([128, NB], F32, tag="ps", name=f"ps0_{rt}") for rt in range(6)]

            def lora_mm(rb, k):
                nc.tensor.matmul(
                    ps_l[rb][:],
                    a_sb[:, k, :],
                    xt[:, k, rb * NB : (rb + 1) * NB],
                    start=(k == 0),
                    stop=(k == KT - 1),
                )

            def main_mm(rt, k):
                nc.tensor.matmul(
                    ps0[rt][:],
                    xt[:, k, rt * 128 : (rt + 1) * 128],
                    wt0[:, k, :],
                    start=(k == 0),
                    stop=False,
                )

            for k in range(KT):
                for rb in range(NRB):
                    lora_mm(rb, k)
                for rt in range(6):
                    main_mm(rt, k)
            # gate multiply, rounded to bf16 for the augmented matmul
            for rb in range(NRB):
                nc.vector.tensor_mul(
                    lora_aug[0:R_LORA, rb * NB : (rb + 1) * NB],
                    ps_l[rb][:],
                    g_sb[:, rb * NB : (rb + 1) * NB],
                )
            for rt in range(6):
                emit_aug(ps0[rt], rt, 0, start=False, stop=True)
                emit_epilogue(ps0[rt], rt, 0)
            for rt in (6, 7):
                ps = psum.tile([128, NB], F32, tag="ps", name=f"ps0b_{rt}")
                for k in range(KT):
                    nc.tensor.matmul(
                        ps[:],
                        xt[:, k, rt * 128 : (rt + 1) * 128],
                        wt0[:, k, :],
                        start=(k == 0),
                        stop=False,
                    )
                emit_aug(ps, rt, 0, start=False, stop=True)
                emit_epilogue(ps, rt, 0)

            # ---- steady state: ob = 1..7 ---------------------------------------
            for ob in range(1, OB):
                wt = wpool.tile([128, KT, NB], BF16, tag="wt", name=f"wt{ob}")
                kw = KT // wt_split
                for h in range(wt_split):
                    nc.sync.dma_start(
                        wt[:, h * kw : (h + 1) * kw, :],
                        w_d.ap()[:, ob, h * kw : (h + 1) * kw, :],
                    )
                for rt in range(RT):
                    last_tile = ob == OB - 1 and rt == RT - 1
                    if last_tile:
                        # Final tile as [tail_split, NB - tail_split] column
                        # sub-tiles: the last store's copy+DMA chain starts
                        # right after a short sub-tile instead of a full one.
                        splits = [(0, tail_split), (tail_split, NB)]
                        for h, (c0, c1) in enumerate(splits):
                            ph = psum.tile(
                                [128, c1 - c0], F32, tag="ps", name=f"ps{ob}_{rt}_h{h}"
                            )
                            ocs = slice(ob * NB + c0, ob * NB + c1)
                            nc.tensor.matmul(
                                ph[:],
                                lora_aug[:, rt * 128 : (rt + 1) * 128],
                                rhs_sb[:, ocs],
                                start=True,
                                stop=False,
                            )
                            for k in range(KT):
                                nc.tensor.matmul(
                                    ph[:],
                                    xt[:, k, rt * 128 : (rt + 1) * 128],
                                    wt[:, k, c0:c1],
                                    start=False,
                                    stop=(k == KT - 1),
                                )
                            o_sb = opool.tile(
                                [128, c1 - c0], BF16, tag="o_sb", name=f"o_{ob}_{rt}_h{h}"
                            )
                            orow = out_d.ap()[rt * 128 : (rt + 1) * 128, ocs]
                            nc.vector.tensor_copy(o_sb[:], ph[:])
                            (nc.scalar if h == 0 else nc.sync).dma_start(orow[:], o_sb[:])
                        continue
                    ps = psum.tile([128, NB], F32, tag="ps", name=f"ps{ob}_{rt}")
                    if AUG_FIRST:
                        emit_aug(ps, rt, ob, start=True, stop=False)
                    for k in range(KT):
                        nc.tensor.matmul(
                            ps[:],
                            xt[:, k, rt * 128 : (rt + 1) * 128],
                            wt[:, k, :],
                            start=(not AUG_FIRST and k == 0),
                            stop=(AUG_FIRST and k == KT - 1),
                        )
                    if not AUG_FIRST:
                        emit_aug(ps, rt, ob, start=False, stop=True)
                    emit_epilogue(ps, rt, ob)

    nc.compile()
    return nc


_NC_CACHE = None


def _get_nc():
    global _NC_CACHE
    if _NC_CACHE is None:
        _NC_CACHE = _build()
    return _NC_CACHE


def _bf16(a: np.ndarray) -> np.ndarray:
    return np.ascontiguousarray(a, dtype=np.float32).astype(NP_BF16)


def _prep_in_maps(x, W, b, A, B_lora, gates, alpha):
    x = np.asarray(x, dtype=np.float32).reshape(ROWS, D_IN)
    W = np.asarray(W, dtype=np.float32)
    b = np.asarray(b, dtype=np.float32)
    A_last = np.asarray(A, dtype=np.float32)[-1]          # [D_IN, 16]
    B_last = np.asarray(B_lora, dtype=np.float32)[-1]     # [16, D_OUT]
    g_last = np.asarray(gates, dtype=np.float32)[-1].reshape(ROWS)
    alpha_f = float(np.asarray(alpha))

    # W.T packed as [ki, ob, ko, o'] so each o-block DMA is one contiguous
    # run per partition.
    wt = W.T.reshape(KT, 128, OB, NB).transpose(1, 2, 0, 3)
    w_pre = _bf16(wt)

    a_pre = _bf16(A_last.reshape(KT, 128, R_LORA).transpose(1, 0, 2))
    aug = np.concatenate([alpha_f * B_last, b[None, :]], axis=0)  # [17, D_OUT]
    aug_pre = _bf16(aug)
    ones_row = np.ones((1, R_CORE), dtype=NP_BF16)

    # rows rotated by ROT per core (see module docstring); self-inverse
    perm = np.concatenate([np.arange(ROT, R_CORE), np.arange(ROT)])

    in_maps = []
    for c in range(N_CORES):
        rows = slice(c * R_CORE, (c + 1) * R_CORE)
        xs = x[rows][perm]                                # [R_CORE, D_IN]
        xt = xs.T.reshape(KT, 128, R_CORE).transpose(1, 0, 2)
        x_pre = _bf16(xt)
        g_rep = np.ascontiguousarray(
            np.broadcast_to(g_last[rows][perm][None, :], (R_LORA, R_CORE))
        ).astype(np.float32)
        in_maps.append(
            {
                "xt": x_pre,
                "wt": w_pre,
                "a_lora": a_pre,
                "aug_rhs": aug_pre,
                "g_rep": g_rep,
                "ones_row": ones_row,
            }
        )
    return in_maps


def run(inputs: dict, trace: bool = False, trace_cores=None):
    """Run the kernel; returns (full_output, BassKernelResults)."""
    nc = _get_nc()
    in_maps = _prep_in_maps(**inputs)
    res = run_bass_kernel_spmd(
        nc,
        in_maps,
        core_ids=list(range(N_CORES)),
        trace=trace,
        trace_cores=trace_cores,
    )
    inv = np.concatenate([np.arange(R_CORE - ROT, R_CORE), np.arange(R_CORE - ROT)])
    outs = [np.asarray(r["out"], dtype=np.float32)[inv] for r in res.results]
    out = np.concatenate(outs, axis=0)
    return out.reshape(B, S, D_OUT).astype(np.float32), res


def kernel(**inputs) -> np.ndarray:
    out, _ = run(inputs, trace=False)
    return out


# revision 12
# speedup vs baseline: 1.0144x; 1.0144x over previous
"""Trainium2 Bass kernel for ClassLinearWithLORA (moe_routing).

Computes out = x @ W.T + b + gates[-1] * (alpha * (x @ A[-1]) @ B_lora[-1])
(the torch loop overwrites out_lora each class iteration, so only the last
class adapter contributes).

Strategy:
  - Data-parallel shard of the 8192 (B*S) rows across 8 NeuronCores
    (1024 rows/core); W/b and the rank-16 LoRA stacks are replicated.
  - Matmuls run in bf16 (1 cycle/row on the PE, same rate as fp32r, but
    HALF the HBM/DMA traffic, which removes PE starvation stalls). PSUM
    accumulation stays fp32; the output is stored bf16 and widened on host.
  - Formulation: psum[r128, o512] = sum_k xT[k][:, r].T @ WT[k][:, o]
    accumulated over 8 K-tiles, plus ONE augmented K=17 matmul per tile
    that adds both the LoRA rank-16 update and the bias:
       lhsT_aug = [ (g * (x @ A)).T ; ones ]  (17 x r)
       rhs_aug  = [ alpha * B_lora[-1] ; b ]  (17 x o)
  - Startup: the per-core rows are rotated by 512 on the host so the first
    K-chunk of xT splits into a tiny 32KB piece (Pool/SWDGE ring, which
    does not contend for the shared HWDGE descriptor generator) plus one
    big ACT-ring piece; wt block 0's first K-slice leads the SP ring.
    All three first DMAs land ~2.6-3.0us and the PE starts at ~2.9us.
  - Tail: the final tile runs as [384, 128] column sub-tiles so the last
    store's copy+DMA chain is as short as possible after the PE finishes.
"""

import numpy as np
import ml_dtypes

import concourse.bacc as bacc
import concourse.mybir as mybir
import concourse.tile as tile
from concourse.bass_utils import run_bass_kernel_spmd

F32 = mybir.dt.float32
BF16 = mybir.dt.bfloat16
NP_BF16 = ml_dtypes.bfloat16

N_CORES = 8
B, S, D_IN, D_OUT, R_LORA = 4, 2048, 1024, 4096, 16
ROWS = B * S                  # 8192
R_CORE = ROWS // N_CORES      # 1024 rows per core
KT = D_IN // 128              # 8 K-tiles of 128
NB = 512                      # moving free dim per matmul (PSUM bank limit)
OB = D_OUT // NB              # 8 output blocks
RT = R_CORE // 128            # 8 row tiles per core
KA = R_LORA + 1               # augmented contraction (16 LoRA + 1 bias)
ROT = 512                     # host-side row rotation (see module docstring)


AUG_FIRST = True

def _build(
    xt_chunks: int = 8,
    xt_engine: str = "scalar",
    wt_bufs: int = 3,
    psum_bufs: int = 8,
    out_bufs: int = 4,
    wt0_split: int = 8,
    wt_split: int = 2,
    tail_split: int = 384,
):
    nc = bacc.Bacc(None, target_bir_lowering=False)

    x_d = nc.dram_tensor("xt", [128, KT, R_CORE], BF16, kind="ExternalInput")
    w_d = nc.dram_tensor("wt", [128, OB, KT, NB], BF16, kind="ExternalInput")
    a_d = nc.dram_tensor("a_lora", [128, KT, R_LORA], BF16, kind="ExternalInput")
    rhs_d = nc.dram_tensor("aug_rhs", [KA, D_OUT], BF16, kind="ExternalInput")
    g_d = nc.dram_tensor("g_rep", [R_LORA, R_CORE], F32, kind="ExternalInput")
    one_d = nc.dram_tensor("ones_row", [1, R_CORE], BF16, kind="ExternalInput")
    out_d = nc.dram_tensor("out", [R_CORE, D_OUT], BF16, kind="ExternalOutput")

    with tile.TileContext(nc) as tc:
        with (
            tc.tile_pool(name="resident", bufs=1) as res,
            tc.tile_pool(name="wpool", bufs=wt_bufs) as wpool,
            tc.tile_pool(name="opool", bufs=out_bufs) as opool,
            tc.tile_pool(name="psum", bufs=psum_bufs, space="PSUM") as psum,
        ):
            # ---- resident loads -------------------------------------------------
            # Ring assignment at t=0 (every early DMA pays ~630ns on the
            # shared HWDGE device, so each ring leads with what the PE needs
            # first):
            #   SP:   wt0 k-slice 0, a_lora, wt0 k-slices 1..7, steady wt
            #   ACT:  xt k0 cols 128:1024, xt chunks k=1..7, output stores
            #   Pool: xt k0 cols 0:128 (SWDGE, no HWDGE contention), g/rhs/ones
            ld = getattr(nc, xt_engine)
            a_sb = res.tile([128, KT, R_LORA], BF16)
            nc.sync.dma_start(a_sb[:], a_d.ap())
            wt0 = wpool.tile([128, KT, NB], BF16, tag="wt")
            kh = KT // wt0_split
            for h in range(wt0_split):
                nc.sync.dma_start(
                    wt0[:, h * kh : (h + 1) * kh, :],
                    w_d.ap()[:, 0, h * kh : (h + 1) * kh, :],
                )
            # g/rhs/ones are not consumed until the gate multiply and first
            # aug matmul (~15us in) — load them after wt block 0
            g_sb = res.tile([R_LORA, R_CORE], F32)
            nc.sync.dma_start(g_sb[:], g_d.ap())
            rhs_sb = res.tile([KA, D_OUT], BF16)
            nc.sync.dma_start(rhs_sb[:], rhs_d.ap())
            lora_aug = res.tile([KA, R_CORE], BF16)
            nc.sync.dma_start(lora_aug[R_LORA : R_LORA + 1, :], one_d.ap())
            xt = res.tile([128, KT, R_CORE], BF16)
            hr = R_CORE // 2
            ld.dma_start(xt[:, 0, 0:hr], x_d.ap()[:, 0, 0:hr])
            ld.dma_start(xt[:, 0, hr:R_CORE], x_d.ap()[:, 0, hr:R_CORE])
            kc = KT // xt_chunks
            for k in range(1, xt_chunks):
                ld.dma_start(
                    xt[:, k * kc : (k + 1) * kc, :],
                    x_d.ap()[:, k * kc : (k + 1) * kc, :],
                )

            def emit_epilogue(ps, rt, ob):
                """Close psum tile: copy to SBUF, then store. For the last
                o-block, split copy+store in halves across both HWDGE rings
                (the SP ring is load-free by then) to shorten the tail chain."""
                o_sb = opool.tile([128, NB], BF16, tag="o_sb", name=f"o_{ob}_{rt}")
                orow = out_d.ap()[rt * 128 : (rt + 1) * 128, ob * NB : (ob + 1) * NB]
                if ob == OB - 1:
                    h = NB // 2
                    nc.vector.tensor_copy(o_sb[:, 0:h], ps[:, 0:h])
                    nc.scalar.dma_start(orow[:, 0:h], o_sb[:, 0:h])
                    nc.vector.tensor_copy(o_sb[:, h:NB], ps[:, h:NB])
                    nc.sync.dma_start(orow[:, h:NB], o_sb[:, h:NB])
                else:
                    nc.vector.tensor_copy(o_sb[:], ps[:])
                    nc.scalar.dma_start(orow[:], o_sb[:])

            def emit_aug(ps, rt, ob, start, stop):
                # rank-16 LoRA update + bias in one K=17 matmul
                nc.tensor.matmul(
                    ps[:],
                    lora_aug[:, rt * 128 : (rt + 1) * 128],
                    rhs_sb[:, ob * NB : (ob + 1) * NB],
                    start=start,
                    stop=stop,
                )

            # ---- prologue: ob=0 interleaved with the LoRA first matmul ---------
            # Per K-chunk: 2 lora matmuls plus 6 of the 8 ob=0 row tiles
            # (2 lora + 6 main psum tiles = 8 banks); rt=6,7 run densely
            # afterwards. k=0 order follows DMA arrival: cols 0:128 (Pool,
            # ~2.8us) -> 128:1024 (ACT, ~3.0us) -> a_lora (SP, ~3.6us).
            NRB = R_CORE // NB  # lora row blocks
            ps_l = [psum.tile([R_LORA, NB], F32, tag="ps", name=f"psl{rb}") for rb in range(NRB)]
            ps0 = [psum.tile([128, NB], F32, tag="ps", name=f"ps0_{rt}") for rt in range(6)]

            def lora_mm(rb, k):
                nc.tensor.matmul(
                    ps_l[rb][:],
                    a_sb[:, k, :],
                    xt[:, k, rb * NB : (rb + 1) * NB],
                    start=(k == 0),
                    stop=(k == KT - 1),
                )

            def main_mm(rt, k):
                nc.tensor.matmul(
                    ps0[rt][:],
                    xt[:, k, rt * 128 : (rt + 1) * 128],
                    wt0[:, k, :],
                    start=(k == 0),
                    stop=False,
                )

            for k in range(KT):
                for rb in range(NRB):
                    lora_mm(rb, k)
                for rt in range(6):
                    main_mm(rt, k)
            # gate multiply, rounded to bf16 for the augmented matmul
            for rb in range(NRB):
                nc.vector.tensor_mul(
                    lora_aug[0:R_LORA, rb * NB : (rb + 1) * NB],
                    ps_l[rb][:],
                    g_sb[:, rb * NB : (rb + 1) * NB],
                )
            for rt in range(6):
                emit_aug(ps0[rt], rt, 0, start=False, stop=True)
                emit_epilogue(ps0[rt], rt, 0)
            for rt in (6, 7):
                ps = psum.tile([128, NB], F32, tag="ps", name=f"ps0b_{rt}")
                for k in range(KT):
                    nc.tensor.matmul(
                        ps[:],
                        xt[:, k, rt * 128 : (rt + 1) * 128],
                        wt0[:, k, :],
                        start=(k == 0),
                        stop=False,
                    )
                emit_aug(ps, rt, 0, start=False, stop=True)
                emit_epilogue(ps, rt, 0)

            # ---- steady state: ob = 1..7 ---------------------------------------
            for ob in range(1, OB):
                wt = wpool.tile([128, KT, NB], BF16, tag="wt", name=f"wt{ob}")
                kw = KT // wt_split
                for h in range(wt_split):
                    nc.sync.dma_start(
                        wt[:, h * kw : (h + 1) * kw, :],
                        w_d.ap()[:, ob, h * kw : (h + 1) * kw, :],
                    )
                for rt in range(RT):
                    last_tile = ob == OB - 1 and rt == RT - 1
                    if last_tile:
                        # Final tile as [tail_split, NB - tail_split] column
                        # sub-tiles: the last store's copy+DMA chain starts
                        # right after a short sub-tile instead of a full one.
                        splits = [(0, tail_split), (tail_split, NB)]
                        for h, (c0, c1) in enumerate(splits):
                            ph = psum.tile(
                                [128, c1 - c0], F32, tag="ps", name=f"ps{ob}_{rt}_h{h}"
                            )
                            ocs = slice(ob * NB + c0, ob * NB + c1)
                            nc.tensor.matmul(
                                ph[:],
                                lora_aug[:, rt * 128 : (rt + 1) * 128],
                                rhs_sb[:, ocs],
                                start=True,
                                stop=False,
                            )
                            for k in range(KT):
                                nc.tensor.matmul(
                                    ph[:],
                                    xt[:, k, rt * 128 : (rt + 1) * 128],
                                    wt[:, k, c0:c1],
                                    start=False,
                                    stop=(k == KT - 1),
                                )
                            o_sb = opool.tile(
                                [128, c1 - c0], BF16, tag="o_sb", name=f"o_{ob}_{rt}_h{h}"
                            )
                            orow = out_d.ap()[rt * 128 : (rt + 1) * 128, ocs]
                            nc.vector.tensor_copy(o_sb[:], ph[:])
                            (nc.scalar if h == 0 else nc.sync).dma_start(orow[:], o_sb[:])
                        continue
                    ps = psum.tile([128, NB], F32, tag="ps", name=f"ps{ob}_{rt}")
                    if AUG_FIRST:
                        emit_aug(ps, rt, ob, start=True, stop=False)
                    for k in range(KT):
                        nc.tensor.matmul(
                            ps[:],
                            xt[:, k, rt * 128 : (rt + 1) * 128],
                            wt[:, k, :],
                            start=(not AUG_FIRST and k == 0),
                            stop=(AUG_FIRST and k == KT - 1),
                        )
                    if not AUG_FIRST:
                        emit_aug(ps, rt, ob, start=False, stop=True)
                    emit_epilogue(ps, rt, ob)

    nc.compile()
    return nc


_NC_CACHE = None


def _get_nc():
    global _NC_CACHE
    if _NC_CACHE is None:
        _NC_CACHE = _build()
    return _NC_CACHE


def _bf16(a: np.ndarray) -> np.ndarray:
    return np.ascontiguousarray(a, dtype=np.float32).astype(NP_BF16)


def _prep_in_maps(x, W, b, A, B_lora, gates, alpha):
    x = np.asarray(x, dtype=np.float32).reshape(ROWS, D_IN)
    W = np.asarray(W, dtype=np.float32)
    b = np.asarray(b, dtype=np.float32)
    A_last = np.asarray(A, dtype=np.float32)[-1]          # [D_IN, 16]
    B_last = np.asarray(B_lora, dtype=np.float32)[-1]     # [16, D_OUT]
    g_last = np.asarray(gates, dtype=np.float32)[-1].reshape(ROWS)
    alpha_f = float(np.asarray(alpha))

    # W.T packed as [ki, ob, ko, o'] so each o-block DMA is one contiguous
    # run per partition.
    wt = W.T.reshape(KT, 128, OB, NB).transpose(1, 2, 0, 3)
    w_pre = _bf16(wt)

    a_pre = _bf16(A_last.reshape(KT, 128, R_LORA).transpose(1, 0, 2))
    aug = np.concatenate([alpha_f * B_last, b[None, :]], axis=0)  # [17, D_OUT]
    aug_pre = _bf16(aug)
    ones_row = np.ones((1, R_CORE), dtype=NP_BF16)

    # rows rotated by ROT per core (see module docstring); self-inverse
    perm = np.concatenate([np.arange(ROT, R_CORE), np.arange(ROT)])

    in_maps = []
    for c in range(N_CORES):
        rows = slice(c * R_CORE, (c + 1) * R_CORE)
        xs = x[rows][perm]                                # [R_CORE, D_IN]
        xt = xs.T.reshape(KT, 128, R_CORE).transpose(1, 0, 2)
        x_pre = _bf16(xt)
        g_rep = np.ascontiguousarray(
            np.broadcast_to(g_last[rows][perm][None, :], (R_LORA, R_CORE))
        ).astype(np.float32)
        in_maps.append(
            {
                "xt": x_pre,
                "wt": w_pre,
                "a_lora": a_pre,
                "aug_rhs": aug_pre,
                "g_rep": g_rep,
                "ones_row": ones_row,
            }
        )
    return in_maps


def run(inputs: dict, trace: bool = False, trace_cores=None):
    """Run the kernel; returns (full_output, BassKernelResults)."""
    nc = _get_nc()
    in_maps = _prep_in_maps(**inputs)
    res = run_bass_kernel_spmd(
        nc,
        in_maps,
        core_ids=list(range(N_CORES)),
        trace=trace,
        trace_cores=trace_cores,
    )
    inv = np.concatenate([np.arange(R_CORE - ROT, R_CORE), np.arange(R_CORE - ROT)])
    outs = [np.asarray(r["out"], dtype=np.float32)[inv] for r in res.results]
    out = np.concatenate(outs, axis=0)
    return out.reshape(B, S, D_OUT).astype(np.float32), res


def kernel(**inputs) -> np.ndarray:
    out, _ = run(inputs, trace=False)
    return out


# revision 13
# speedup vs baseline: 1.0150x; 1.0006x over previous
"""Trainium2 Bass kernel for ClassLinearWithLORA (moe_routing).

Computes out = x @ W.T + b + gates[-1] * (alpha * (x @ A[-1]) @ B_lora[-1])
(the torch loop overwrites out_lora each class iteration, so only the last
class adapter contributes).

Strategy:
  - Data-parallel shard of the 8192 (B*S) rows across 8 NeuronCores
    (1024 rows/core); W/b and the rank-16 LoRA stacks are replicated.
  - Matmuls run in bf16 (1 cycle/row on the PE, same rate as fp32r, but
    HALF the HBM/DMA traffic, which removes PE starvation stalls). PSUM
    accumulation stays fp32; the output is stored bf16 and widened on host.
  - Formulation: psum[r128, o512] = sum_k xT[k][:, r].T @ WT[k][:, o]
    accumulated over 8 K-tiles, plus ONE augmented K=17 matmul per tile
    that adds both the LoRA rank-16 update and the bias:
       lhsT_aug = [ (g * (x @ A)).T ; ones ]  (17 x r)
       rhs_aug  = [ alpha * B_lora[-1] ; b ]  (17 x o)
  - Startup: the per-core rows are rotated by 512 on the host so the first
    K-chunk of xT splits into a tiny 32KB piece (Pool/SWDGE ring, which
    does not contend for the shared HWDGE descriptor generator) plus one
    big ACT-ring piece; wt block 0's first K-slice leads the SP ring.
    All three first DMAs land ~2.6-3.0us and the PE starts at ~2.9us.
  - Tail: the final tile runs as [384, 128] column sub-tiles so the last
    store's copy+DMA chain is as short as possible after the PE finishes.
"""

import numpy as np
import ml_dtypes

import concourse.bacc as bacc
import concourse.mybir as mybir
import concourse.tile as tile
from concourse.bass_utils import run_bass_kernel_spmd

F32 = mybir.dt.float32
BF16 = mybir.dt.bfloat16
NP_BF16 = ml_dtypes.bfloat16

N_CORES = 8
B, S, D_IN, D_OUT, R_LORA = 4, 2048, 1024, 4096, 16
ROWS = B * S                  # 8192
R_CORE = ROWS // N_CORES      # 1024 rows per core
KT = D_IN // 128              # 8 K-tiles of 128
NB = 512                      # moving free dim per matmul (PSUM bank limit)
OB = D_OUT // NB              # 8 output blocks
RT = R_CORE // 128            # 8 row tiles per core
KA = R_LORA + 1               # augmented contraction (16 LoRA + 1 bias)
ROT = 512                     # host-side row rotation (see module docstring)


AUG_FIRST = True

def _build(
    xt_chunks: int = 8,
    xt_engine: str = "scalar",
    wt_bufs: int = 3,
    psum_bufs: int = 8,
    out_bufs: int = 4,
    wt0_split: int = 8,
    wt_split: int = 2,
    tail_split: int = 384,
):
    nc = bacc.Bacc(None, target_bir_lowering=False)

    x_d = nc.dram_tensor("xt", [128, KT, R_CORE], BF16, kind="ExternalInput")
    w_d = nc.dram_tensor("wt", [128, OB, KT, NB], BF16, kind="ExternalInput")
    a_d = nc.dram_tensor("a_lora", [128, KT, R_LORA], BF16, kind="ExternalInput")
    rhs_d = nc.dram_tensor("aug_rhs", [KA, D_OUT], BF16, kind="ExternalInput")
    g_d = nc.dram_tensor("g_rep", [R_LORA, R_CORE], F32, kind="ExternalInput")
    one_d = nc.dram_tensor("ones_row", [1, R_CORE], BF16, kind="ExternalInput")
    out_d = nc.dram_tensor("out", [R_CORE, D_OUT], BF16, kind="ExternalOutput")

    with tile.TileContext(nc) as tc:
        with (
            tc.tile_pool(name="resident", bufs=1) as res,
            tc.tile_pool(name="wpool", bufs=wt_bufs) as wpool,
            tc.tile_pool(name="opool", bufs=out_bufs) as opool,
            tc.tile_pool(name="psum", bufs=psum_bufs, space="PSUM") as psum,
        ):
            # ---- resident loads -------------------------------------------------
            # Ring assignment at t=0 (every early DMA pays ~630ns on the
            # shared HWDGE device, so each ring leads with what the PE needs
            # first):
            #   SP:   wt0 k-slice 0, a_lora, wt0 k-slices 1..7, steady wt
            #   ACT:  xt k0 cols 128:1024, xt chunks k=1..7, output stores
            #   Pool: xt k0 cols 0:128 (SWDGE, no HWDGE contention), g/rhs/ones
            ld = getattr(nc, xt_engine)
            a_sb = res.tile([128, KT, R_LORA], BF16)
            nc.sync.dma_start(a_sb[:], a_d.ap())
            wt0 = wpool.tile([128, KT, NB], BF16, tag="wt")
            kh = KT // wt0_split
            for h in range(wt0_split):
                nc.sync.dma_start(
                    wt0[:, h * kh : (h + 1) * kh, :],
                    w_d.ap()[:, 0, h * kh : (h + 1) * kh, :],
                )
            # g/rhs/ones are not consumed until the gate multiply and first
            # aug matmul (~15us in) — load them after wt block 0
            g_sb = res.tile([R_LORA, R_CORE], F32)
            nc.sync.dma_start(g_sb[:], g_d.ap())
            rhs_sb = res.tile([KA, D_OUT], BF16)
            nc.sync.dma_start(rhs_sb[:], rhs_d.ap())
            lora_aug = res.tile([KA, R_CORE], BF16)
            nc.sync.dma_start(lora_aug[R_LORA : R_LORA + 1, :], one_d.ap())
            xt = res.tile([128, KT, R_CORE], BF16)
            hr = R_CORE // 2
            ld.dma_start(xt[:, 0, 0:hr], x_d.ap()[:, 0, 0:hr])
            ld.dma_start(xt[:, 0, hr:R_CORE], x_d.ap()[:, 0, hr:R_CORE])
            kc = KT // xt_chunks
            for k in range(1, xt_chunks):
                ld.dma_start(
                    xt[:, k * kc : (k + 1) * kc, :],
                    x_d.ap()[:, k * kc : (k + 1) * kc, :],
                )

            def emit_epilogue(ps, rt, ob):
                """Close psum tile: copy to SBUF, then store. For the last
                o-block, split copy+store in halves across both HWDGE rings
                (the SP ring is load-free by then) to shorten the tail chain."""
                o_sb = opool.tile([128, NB], BF16, tag="o_sb", name=f"o_{ob}_{rt}")
                orow = out_d.ap()[rt * 128 : (rt + 1) * 128, ob * NB : (ob + 1) * NB]
                if ob == OB - 1:
                    h = NB // 2
                    nc.vector.tensor_copy(o_sb[:, 0:h], ps[:, 0:h])
                    nc.scalar.dma_start(orow[:, 0:h], o_sb[:, 0:h])
                    nc.vector.tensor_copy(o_sb[:, h:NB], ps[:, h:NB])
                    nc.sync.dma_start(orow[:, h:NB], o_sb[:, h:NB])
                else:
                    nc.vector.tensor_copy(o_sb[:], ps[:])
                    nc.scalar.dma_start(orow[:], o_sb[:])

            def emit_aug(ps, rt, ob, start, stop):
                # rank-16 LoRA update + bias in one K=17 matmul
                nc.tensor.matmul(
                    ps[:],
                    lora_aug[:, rt * 128 : (rt + 1) * 128],
                    rhs_sb[:, ob * NB : (ob + 1) * NB],
                    start=start,
                    stop=stop,
                )

            # ---- prologue: ob=0 interleaved with the LoRA first matmul ---------
            # Per K-chunk: 2 lora matmuls plus 6 of the 8 ob=0 row tiles
            # (2 lora + 6 main psum tiles = 8 banks); rt=6,7 run densely
            # afterwards. k=0 order follows DMA arrival: cols 0:128 (Pool,
            # ~2.8us) -> 128:1024 (ACT, ~3.0us) -> a_lora (SP, ~3.6us).
            NRB = R_CORE // NB  # lora row blocks
            ps_l = [psum.tile([R_LORA, NB], F32, tag="ps", name=f"psl{rb}") for rb in range(NRB)]
            ps0 = [psum.tile([128, NB], F32, tag="ps", name=f"ps0_{rt}") for rt in range(6)]

            def lora_mm(rb, k):
                nc.tensor.matmul(
                    ps_l[rb][:],
                    a_sb[:, k, :],
                    xt[:, k, rb * NB : (rb + 1) * NB],
                    start=(k == 0),
                    stop=(k == KT - 1),
                )

            def main_mm(rt, k):
                nc.tensor.matmul(
                    ps0[rt][:],
                    xt[:, k, rt * 128 : (rt + 1) * 128],
                    wt0[:, k, :],
                    start=(k == 0),
                    stop=False,
                )

            # k=0 order follows DMA arrival: lora rb0 (a_lora + xt half 0),
            # then the rt0-3 mains (wt block 0 k-slice 0) fill the window
            # until xt half 1 lands for lora rb1 and the rt4-5 mains.
            lora_mm(0, 0)
            for rt in (0, 1, 2, 3):
                main_mm(rt, 0)
            lora_mm(1, 0)
            for rt in (4, 5):
                main_mm(rt, 0)
            for k in range(1, KT):
                for rb in range(NRB):
                    lora_mm(rb, k)
                for rt in range(6):
                    main_mm(rt, k)
            # gate multiply, rounded to bf16 for the augmented matmul
            for rb in range(NRB):
                nc.vector.tensor_mul(
                    lora_aug[0:R_LORA, rb * NB : (rb + 1) * NB],
                    ps_l[rb][:],
                    g_sb[:, rb * NB : (rb + 1) * NB],
                )
            for rt in range(6):
                emit_aug(ps0[rt], rt, 0, start=False, stop=True)
                emit_epilogue(ps0[rt], rt, 0)
            for rt in (6, 7):
                ps = psum.tile([128, NB], F32, tag="ps", name=f"ps0b_{rt}")
                for k in range(KT):
                    nc.tensor.matmul(
                        ps[:],
                        xt[:, k, rt * 128 : (rt + 1) * 128],
                        wt0[:, k, :],
                        start=(k == 0),
                        stop=False,
                    )
                emit_aug(ps, rt, 0, start=False, stop=True)
                emit_epilogue(ps, rt, 0)

            # ---- steady state: ob = 1..7 ---------------------------------------
            for ob in range(1, OB):
                wt = wpool.tile([128, KT, NB], BF16, tag="wt", name=f"wt{ob}")
                kw = KT // wt_split
                for h in range(wt_split):
                    nc.sync.dma_start(
                        wt[:, h * kw : (h + 1) * kw, :],
                        w_d.ap()[:, ob, h * kw : (h + 1) * kw, :],
                    )
                for rt in range(RT):
                    last_tile = ob == OB - 1 and rt == RT - 1
                    if last_tile:
                        # Final tile as [tail_split, NB - tail_split] column
                        # sub-tiles: the last store's copy+DMA chain starts
                        # right after a short sub-tile instead of a full one.
                        splits = [(0, tail_split), (tail_split, NB)]
                        for h, (c0, c1) in enumerate(splits):
                            ph = psum.tile(
                                [128, c1 - c0], F32, tag="ps", name=f"ps{ob}_{rt}_h{h}"
                            )
                            ocs = slice(ob * NB + c0, ob * NB + c1)
                            nc.tensor.matmul(
                                ph[:],
                                lora_aug[:, rt * 128 : (rt + 1) * 128],
                                rhs_sb[:, ocs],
                                start=True,
                                stop=False,
                            )
                            for k in range(KT):
                                nc.tensor.matmul(
                                    ph[:],
                                    xt[:, k, rt * 128 : (rt + 1) * 128],
                                    wt[:, k, c0:c1],
                                    start=False,
                                    stop=(k == KT - 1),
                                )
                            o_sb = opool.tile(
                                [128, c1 - c0], BF16, tag="o_sb", name=f"o_{ob}_{rt}_h{h}"
                            )
                            orow = out_d.ap()[rt * 128 : (rt + 1) * 128, ocs]
                            nc.vector.tensor_copy(o_sb[:], ph[:])
                            (nc.scalar if h == 0 else nc.sync).dma_start(orow[:], o_sb[:])
                        continue
                    ps = psum.tile([128, NB], F32, tag="ps", name=f"ps{ob}_{rt}")
                    if AUG_FIRST:
                        emit_aug(ps, rt, ob, start=True, stop=False)
                    for k in range(KT):
                        nc.tensor.matmul(
                            ps[:],
                            xt[:, k, rt * 128 : (rt + 1) * 128],
                            wt[:, k, :],
                            start=(not AUG_FIRST and k == 0),
                            stop=(AUG_FIRST and k == KT - 1),
                        )
                    if not AUG_FIRST:
                        emit_aug(ps, rt, ob, start=False, stop=True)
                    emit_epilogue(ps, rt, ob)

    nc.compile()
    return nc


_NC_CACHE = None


def _get_nc():
    global _NC_CACHE
    if _NC_CACHE is None:
        _NC_CACHE = _build()
    return _NC_CACHE


def _bf16(a: np.ndarray) -> np.ndarray:
    return np.ascontiguousarray(a, dtype=np.float32).astype(NP_BF16)


def _prep_in_maps(x, W, b, A, B_lora, gates, alpha):
    x = np.asarray(x, dtype=np.float32).reshape(ROWS, D_IN)
    W = np.asarray(W, dtype=np.float32)
    b = np.asarray(b, dtype=np.float32)
    A_last = np.asarray(A, dtype=np.float32)[-1]          # [D_IN, 16]
    B_last = np.asarray(B_lora, dtype=np.float32)[-1]     # [16, D_OUT]
    g_last = np.asarray(gates, dtype=np.float32)[-1].reshape(ROWS)
    alpha_f = float(np.asarray(alpha))

    # W.T packed as [ki, ob, ko, o'] so each o-block DMA is one contiguous
    # run per partition.
    wt = W.T.reshape(KT, 128, OB, NB).transpose(1, 2, 0, 3)
    w_pre = _bf16(wt)

    a_pre = _bf16(A_last.reshape(KT, 128, R_LORA).transpose(1, 0, 2))
    aug = np.concatenate([alpha_f * B_last, b[None, :]], axis=0)  # [17, D_OUT]
    aug_pre = _bf16(aug)
    ones_row = np.ones((1, R_CORE), dtype=NP_BF16)

    # rows rotated by ROT per core (see module docstring); self-inverse
    perm = np.concatenate([np.arange(ROT, R_CORE), np.arange(ROT)])

    in_maps = []
    for c in range(N_CORES):
        rows = slice(c * R_CORE, (c + 1) * R_CORE)
        xs = x[rows][perm]                                # [R_CORE, D_IN]
        xt = xs.T.reshape(KT, 128, R_CORE).transpose(1, 0, 2)
        x_pre = _bf16(xt)
        g_rep = np.ascontiguousarray(
            np.broadcast_to(g_last[rows][perm][None, :], (R_LORA, R_CORE))
        ).astype(np.float32)
        in_maps.append(
            {
                "xt": x_pre,
                "wt": w_pre,
                "a_lora": a_pre,
                "aug_rhs": aug_pre,
                "g_rep": g_rep,
                "ones_row": ones_row,
            }
        )
    return in_maps


def run(inputs: dict, trace: bool = False, trace_cores=None):
    """Run the kernel; returns (full_output, BassKernelResults)."""
    nc = _get_nc()
    in_maps = _prep_in_maps(**inputs)
    res = run_bass_kernel_spmd(
        nc,
        in_maps,
        core_ids=list(range(N_CORES)),
        trace=trace,
        trace_cores=trace_cores,
    )
    inv = np.concatenate([np.arange(R_CORE - ROT, R_CORE), np.arange(R_CORE - ROT)])
    outs = [np.asarray(r["out"], dtype=np.float32)[inv] for r in res.results]
    out = np.concatenate(outs, axis=0)
    return out.reshape(B, S, D_OUT).astype(np.float32), res


def kernel(**inputs) -> np.ndarray:
    out, _ = run(inputs, trace=False)
    return out
